# revision 68
# baseline (speedup 1.0000x reference)
"""EnhancedAttentionModule Trainium2 kernel.

x: [16, 512, 4096] f32.  Module:
    pooled = mean_n(x)                      # [B, C]
    h  = relu(pooled @ w1.T + b1)           # [B, C/4]
    ca = sigmoid(h @ w2.T + b2)             # [B, C]  (channel attention)
    x_ca = x * ca[:, :, None]
    h2 = BN(w3 @ x_ca + b3); h2 = relu(h2)  # [B, C/4, N]
    sa = sigmoid(w4 @ h2 + b4)              # [B, 1, N] (spatial attention)
    out = x + x_ca * sa = x * (1 + ca*sa)

The problem is HBM-bound on a serial DMA resource (~360 GB/s modeled),
so both wire formats are reduced precision (harness tolerance 2e-2):
  - x is converted to float16 on the host before upload (halves load
    traffic; adds ~2.4e-4 relative error),
  - the output is written as float16 and upcast on the host during the
    gather (halves store traffic).
Per core: 8.4 MB read + 8.4 MB write + ~0.8 MB weights.

Device-side structure:
  - mean divisor folded into w1, BN folded into w3/bias (host); ca
    folded into the w3 matmul weights on device (w3e = w3Ti * ca) so
    x_ca is never materialized.
  - out = x * (1 + ca[c]*sa[n]): the rank-1 product ca (x) sa goes
    straight into PSUM via a K=1 matmul per [128,512] bank; the +1 and
    the multiply fuse into one DVE scalar_tensor_tensor per [128,1024]
    block ((s2raw + 1) * x -> f16), stored immediately (32 block
    stores/core).
  - for j==3 (both batches) and j==2 (batch 0) ACT instead evicts
    s2+1 to f16 SBUF (activation Copy, bias=1.0) and the otherwise
    idle Pool engine does the multiply -- GPSIMD cannot read PSUM on
    real HW, and this keeps DVE production ahead of the DMA drain.
  - each pooled sum is ONE DVE op: scalar_tensor_tensor computes the
    f16 pairwise add of the tile halves with accum_out producing the
    sum directly.  The last tile of each batch is pooled in quarters
    so the MLP can start right after its DMA lands; the MLP
    accumulates all 7 partial vectors via matmul linearity.  All
    large weights (w3Ti, w1nT, w2T, w4) ship as one f16 blob and the
    whole channel-MLP runs in f16.
  - sa pairs interleave with s2/multiply/store emission, each group
    one pair late so PE (in-order) never stalls on the pair's
    sigmoids; the sa PSUM ring is double-buffered (the MLP borrows a
    ps_h2 slot to free the bank).
  - weight DMAs are emitted after the first x-tile load so their
    HWDGE issue latency hides under the big transfer (back-to-back
    small transfers cannot keep the issue pipeline fed).
  - a dummy sigmoid at kernel start pulls both activation-table loads
    off the critical path.
  - matmuls run f16 (x-side) and float32r (s2/sa side), 1 cycle/row.

Sharding: data-parallel over batch. 8 cores x 2 batches each. Weights
replicated. No collectives.
"""

import numpy as np

B, C, N = 16, 512, 4096
CR = C // 4  # 128
P = 128      # partitions
NCORES = 8
BPC = B // NCORES        # batches per core = 2
CCH = C // P             # channel chunks per batch = 4
NB = N // 512            # 512-wide n blocks = 8
NH = N // 1024           # 1024-wide blocks = 4
BN_EPS = 1e-5

# f16 weight blob ([128, HBLOB])
_W3 = 0          # w3Ti as [p, j, m]: cols [0, 512)
_W1 = 512        # w1nT as [p, j, m]: cols [512, 1024)
_W2 = 1024       # w2T: cols [1024, 1536)
_W4H = 1536      # w4T: col 1536
HBLOB = 1537
# f32 weight blob ([128, FBLOB]): small per-partition vectors
_B1 = 0
_B3 = 1
_B2C = 2         # cols [2, 6)
_B4 = 6          # row 0 only
FBLOB = 7

_CACHE = {}


def _build(n_iter=1):
    import concourse.bacc as bacc
    import concourse.tile as tile
    from concourse import mybir

    f32 = mybir.dt.float32
    f32r = mybir.dt.float32r
    f16 = mybir.dt.float16
    AF = mybir.ActivationFunctionType
    AX = mybir.AxisListType
    ALU = mybir.AluOpType

    nc = bacc.Bacc(None)

    xs = nc.dram_tensor("xs", [BPC * C, N], f16, kind="ExternalInput")
    out = nc.dram_tensor("outv", [BPC * C, N], f16, kind="ExternalOutput")
    wbh_d = nc.dram_tensor("wblobh", [P, HBLOB], f16, kind="ExternalInput")
    wbf_d = nc.dram_tensor("wblobf", [P, FBLOB], f32, kind="ExternalInput")
    b2r_d = nc.dram_tensor("b2row", [1, C], f16, kind="ExternalInput")

    xs_t = xs.rearrange("(t p) n -> t p n", p=P)      # 8 tiles [128, 4096]
    out_t = out.rearrange("(t p) n -> t p n", p=P)

    with tile.TileContext(nc) as tc:
        with (
            tc.tile_pool(name="wpool", bufs=1) as wpool,
            tc.tile_pool(name="xpool", bufs=BPC * CCH) as xpool,
            tc.tile_pool(name="small", bufs=6) as small,
            tc.tile_pool(name="ppool", bufs=3) as ppool,
            tc.tile_pool(name="wefpool", bufs=2 * CCH) as wefpool,
            tc.tile_pool(name="h2spool", bufs=6) as h2spool,
            tc.tile_pool(name="sapool", bufs=2) as sapool,
            tc.tile_pool(name="opool", bufs=32) as opool,
            tc.tile_pool(name="ps_h2", bufs=2, space="PSUM") as ps_h2,
            tc.tile_pool(name="ps_sa", bufs=2, space="PSUM") as ps_sa,
            tc.tile_pool(name="ps_s2", bufs=2, space="PSUM") as ps_s2,
        ):
            wbh = wpool.tile([P, HBLOB], f16)
            wbf = wpool.tile([P, FBLOB], f32)
            b2r_sb = wpool.tile([1, C], f16)
            w3Ti_sb = wbh[:, _W3 : _W3 + 512].rearrange("p (j m) -> p j m", j=CCH)
            w1nT_sb = wbh[:, _W1 : _W1 + 512].rearrange("p (j m) -> p j m", j=CCH)
            w2T_sb = wbh[:, _W2 : _W2 + 512]
            b1_sb = wbf[:, _B1 : _B1 + 1]
            b3e_sb = wbf[:, _B3 : _B3 + 1]
            b2c_sb = wbf[:, _B2C : _B2C + CCH]
            b4_sb = wbf[0:1, _B4 : _B4 + 1]
            w4T_sb = wbh[:, _W4H : _W4H + 1]
            one1f = wpool.tile([1, 1], f32)
            nc.vector.memset(one1f, 1.0)
            one1_sb = wpool.tile([1, 1], f16)
            nc.vector.tensor_copy(one1_sb, one1f)
            # dummy sigmoid: forces the activation-table switch (~1.3us)
            # to happen at kernel start while ACT is idle, not mid-MLP
            dummy = wpool.tile([1, 1], f32)
            nc.scalar.activation(dummy, one1f, AF.Sigmoid)
            # sa|ca tiles: cols [0,N) = sa, [N,N+C) = ca_row.  Single
            # row: s2 is computed as the rank-1 product ca (x) sa by a
            # K=1 matmul and the +1 is folded into the final multiply
            # (scalar_tensor_tensor), so no ones rows are needed.
            sa_tiles = []
            for _b in range(BPC):
                sa_t = sapool.tile([1, N + C], f32r, tag="sa")
                sa_tiles.append(sa_t)

            def emit_weight_dmas():
                nc.sync.dma_start(out=wbh, in_=wbh_d[:, :])
                nc.sync.dma_start(out=wbf, in_=wbf_d[:, :])
                nc.sync.dma_start(out=b2r_sb, in_=b2r_d[:, :])

            for _it in range(n_iter):
                # ---- x loads, all emitted up front.  The LAST tile of
                # each batch is split into four quarter-column DMAs so
                # its pooled sum can complete right after the last byte.
                xts = []
                for b in range(BPC):
                    xt = []
                    for j in range(CCH):
                        t = xpool.tile([P, N], f16, tag="xt")
                        xt.append(t)
                        if b == 0 and j == CCH - 1:
                            for q in range(4):
                                nc.sync.dma_start(
                                    out=t[:, q * 1024 : (q + 1) * 1024],
                                    in_=xs_t[j][:, q * 1024 : (q + 1) * 1024],
                                )
                        else:
                            nc.sync.dma_start(out=t, in_=xs_t[b * CCH + j])
                        if _it == 0 and b == 0 and j == 0:
                            emit_weight_dmas()
                    xts.append(xt)

                # ---- pooled sums, two-stage (DVE f16 pair-adds at 2x
                # rate, then a short reduce).  Batch 0 stage 2 on ACT
                # (idle during loads); batch 1 fully on DVE so it isn't
                # queued behind batch 0's ACT chain.  Emitted for both
                # batches here so the reduces track the DMA arrivals.
                parts_by_b = []
                with nc.allow_low_precision(reason="f16 pairwise add; accum f32"):
                    for b in range(BPC):
                        xt = xts[b]
                        parts = []
                        for j in range(CCH - 1):
                            t = xt[j]
                            h1 = ppool.tile([P, 2048], f16, tag="h1")
                            pj = small.tile([P, 1], f16, tag=f"pool_{b}_{j}")
                            nc.vector.scalar_tensor_tensor(
                                h1, t[:, :2048], 0.0, t[:, 2048:],
                                op0=ALU.add, op1=ALU.add, accum_out=pj,
                            )
                            parts.append((j, pj))
                        t = xt[CCH - 1]
                        for q in range(4):
                            qv = t[:, q * 1024 : (q + 1) * 1024]
                            hq = ppool.tile([P, 512], f16, tag="hq")
                            pq = small.tile([P, 1], f16, tag=f"poolq_{b}_{q}")
                            nc.vector.scalar_tensor_tensor(
                                hq, qv[:, :512], 0.0, qv[:, 512:],
                                op0=ALU.add, op1=ALU.add, accum_out=pq,
                            )
                            parts.append((CCH - 1, pq))
                        parts_by_b.append(parts)

                for b in range(BPC):
                    xt = xts[b]
                    # ---- channel attention MLP ----
                    # the MLP psum borrows a ps_h2 ring slot (same shape);
                    # it is consumed well before the second h2 block needs
                    # the slot back, and this frees a PSUM bank so the sa
                    # ring can double-buffer (sa-mm(k+1) must not wait on
                    # sigmoid(k))
                    psum_hca = ps_h2.tile([P, 512], f32, tag="ph2")
                    psum_h = psum_hca[:, 0:1]
                    psum_ca = psum_hca[:, 4:8]
                    parts = parts_by_b[b]
                    for k, (j, pv) in enumerate(parts):
                        nc.tensor.matmul(
                            psum_h,
                            lhsT=w1nT_sb[:, j, :],
                            rhs=pv,
                            start=(k == 0),
                            stop=(k == len(parts) - 1),
                        )
                    h_sb = small.tile([P, 1], f16, tag="h")
                    nc.scalar.activation(h_sb, psum_h, AF.Relu, bias=b1_sb)

                    # ca as per-partition columns [P, CCH] (for the w3 fold)
                    for j in range(CCH):
                        nc.tensor.matmul(
                            psum_ca[:, j : j + 1],
                            lhsT=w2T_sb[:, j * P : (j + 1) * P],
                            rhs=h_sb,
                            start=True,
                            stop=True,
                        )
                    ca_sb = small.tile([P, CCH], f32, tag="ca")
                    for j in range(CCH):
                        nc.scalar.activation(
                            ca_sb[:, j : j + 1],
                            psum_ca[:, j : j + 1],
                            AF.Sigmoid,
                            bias=b2c_sb[:, j : j + 1],
                        )

                    # ca as an augmented row pair: row0 = sigmoid(h@w2T + b2)
                    psum_car = ps_sa.tile([1, C], f32, tag="psa")
                    nc.tensor.matmul(
                        psum_car, lhsT=h_sb, rhs=w2T_sb, start=True, stop=False
                    )
                    nc.tensor.matmul(
                        psum_car, lhsT=one1_sb, rhs=b2r_sb, start=False, stop=True
                    )
                    ca2_sb = sa_tiles[b][:, N : N + C]
                    nc.scalar.activation(ca2_sb[0:1, :], psum_car, AF.Sigmoid)

                    # ---- fold ca into w3 (ACT: out = Copy(in * scale)) ----
                    w3e = []
                    for j in range(CCH):
                        we = wefpool.tile([P, CR], f16, tag="w3e")
                        nc.scalar.activation(
                            we, w3Ti_sb[:, j, :], AF.Copy, scale=ca_sb[:, j : j + 1]
                        )
                        w3e.append(we)

                    # ---- spatial attention + output, interleaved ----
                    sa_sb = sa_tiles[b]

                    def emit_s2_mul_store(nh, b=b, sa_sb=sa_sb, xt=xt,
                                          ca2_sb=ca2_sb):
                        lo = nh * 1024
                        for j in range(CCH):
                            psum_s2 = ps_s2.tile([P, 1024], f32, tag="ps2")
                            for hh in range(2):
                                o = lo + hh * 512
                                nc.tensor.matmul(
                                    psum_s2[:, hh * 512 : (hh + 1) * 512],
                                    lhsT=ca2_sb[:, j * P : (j + 1) * P],
                                    rhs=sa_sb[:, o : o + 512],
                                    start=True,
                                    stop=True,
                                )
                            ob = opool.tile([P, 1024], f16, tag="ob")
                            # out = (ca*sa + 1) * x.  GPSIMD/Pool cannot
                            # read PSUM on real HW, so DVE multiplies from
                            # PSUM directly; for j==3 ACT evicts s2+1 to
                            # f16 SBUF and the otherwise-idle Pool engine
                            # does the multiply.
                            if j == CCH - 1 or (b == 0 and j == CCH - 2):
                                s2f = ppool.tile([P, 1024], f16, tag="s2f")
                                nc.scalar.activation(
                                    s2f, psum_s2, AF.Copy, bias=1.0
                                )
                                nc.gpsimd.tensor_mul(
                                    ob, s2f, xt[j][:, lo : lo + 1024]
                                )
                            elif b == 1 and nh == NH - 1 and j == CCH - 2:
                                # tail-critical block: multiply and store in
                                # 512-halves so the first store issues while
                                # the second half multiplies, and the final
                                # transfer is half-size
                                for hh in range(2):
                                    o = hh * 512
                                    nc.vector.scalar_tensor_tensor(
                                        ob[:, o : o + 512],
                                        psum_s2[:, o : o + 512],
                                        1.0,
                                        xt[j][:, lo + o : lo + o + 512],
                                        op0=ALU.add,
                                        op1=ALU.mult,
                                    )
                                    nc.sync.dma_start(
                                        out=out_t[b * CCH + j][
                                            :, lo + o : lo + o + 512
                                        ],
                                        in_=ob[:, o : o + 512],
                                    )
                                continue
                            else:
                                nc.vector.scalar_tensor_tensor(
                                    ob,
                                    psum_s2,
                                    1.0,
                                    xt[j][:, lo : lo + 1024],
                                    op0=ALU.add,
                                    op1=ALU.mult,
                                )
                            nc.sync.dma_start(
                                out=out_t[b * CCH + j][:, lo : lo + 1024], in_=ob
                            )

                    for nb in range(NB):
                        psum_h2 = ps_h2.tile([P, 512], f32, tag="ph2")
                        for j in range(CCH):
                            nc.tensor.matmul(
                                psum_h2,
                                lhsT=w3e[j],
                                rhs=xt[j][:, nb * 512 : (nb + 1) * 512],
                                start=(j == 0),
                                stop=(j == CCH - 1),
                            )
                        h2s = h2spool.tile([P, 512], f16, tag="h2s")
                        nc.scalar.activation(h2s, psum_h2, AF.Relu, bias=b3e_sb)
                        psum_sa = ps_sa.tile([1, 512], f32, tag="psa")
                        nc.tensor.matmul(
                            psum_sa, lhsT=w4T_sb, rhs=h2s, start=True, stop=True
                        )
                        nc.scalar.activation(
                            sa_sb[0:1, nb * 512 : (nb + 1) * 512],
                            psum_sa,
                            AF.Sigmoid,
                            bias=b4_sb,
                        )
                        # s2/mult/store groups are emitted ONE PAIR LATE:
                        # group g's s2 matmuls depend on pair g's sigmoids,
                        # so emitting them right after pair g stalls PE on
                        # ACT; one pair of h2 work in between hides it.
                        if nb % 2 == 1 and nb >= 3:
                            emit_s2_mul_store((nb - 3) // 2)
                    emit_s2_mul_store(NH - 1)

    nc.finalize()
    return nc


def _get_nc(n_iter=1):
    key = ("nc", n_iter)
    if key not in _CACHE:
        _CACHE[key] = _build(n_iter)
    return _CACHE[key]


def _make_in_maps(inputs):
    x = np.asarray(inputs["x"], dtype=np.float32)
    w1 = np.asarray(inputs["w1"], dtype=np.float32)
    b1 = np.asarray(inputs["b1"], dtype=np.float32)
    w2 = np.asarray(inputs["w2"], dtype=np.float32)
    b2 = np.asarray(inputs["b2"], dtype=np.float32)
    w3 = np.asarray(inputs["w3"], dtype=np.float32)
    b3 = np.asarray(inputs["b3"], dtype=np.float32)
    bn_gamma = np.asarray(inputs["bn_gamma"], dtype=np.float32)
    bn_beta = np.asarray(inputs["bn_beta"], dtype=np.float32)
    bn_mean = np.asarray(inputs["bn_mean"], dtype=np.float32)
    bn_var = np.asarray(inputs["bn_var"], dtype=np.float32)
    w4 = np.asarray(inputs["w4"], dtype=np.float32)
    b4 = np.asarray(inputs["b4"], dtype=np.float32)

    # ---- host-side weight folding (tiny) + f16 wire conversion ----
    inv = bn_gamma / np.sqrt(bn_var + BN_EPS)                   # [CR]
    w1nT = (w1.T / float(N)).reshape(CCH, P, CR).transpose(1, 0, 2)
    w3Ti = (w3.T * inv[None, :]).reshape(CCH, P, CR).transpose(1, 0, 2)
    b3e = b3 * inv + bn_beta - bn_mean * inv

    x16 = np.ascontiguousarray(x.astype(np.float16))
    wbh = np.zeros((P, HBLOB), np.float16)
    wbh[:, _W3 : _W3 + 512] = w3Ti.reshape(P, 512).astype(np.float16)
    wbh[:, _W1 : _W1 + 512] = w1nT.reshape(P, 512).astype(np.float16)
    wbh[:, _W2 : _W2 + 512] = w2.T.astype(np.float16)            # [CR->P, C]
    wbh[:, _W4H] = w4.reshape(CR).astype(np.float16)
    wbf = np.zeros((P, FBLOB), np.float32)
    wbf[:, _B1] = b1
    wbf[:, _B3] = b3e
    wbf[:, _B2C : _B2C + CCH] = b2.reshape(CCH, P).T
    wbf[0, _B4] = b4[0]
    b2row = np.ascontiguousarray(b2.reshape(1, C).astype(np.float16))

    in_maps = []
    for i in range(NCORES):
        in_maps.append(
            {
                "xs": x16[i * BPC : (i + 1) * BPC].reshape(BPC * C, N),
                "wblobh": wbh,
                "wblobf": wbf,
                "b2row": b2row,
            }
        )
    return in_maps


def kernel(**inputs):
    nc = _get_nc()
    in_maps = _make_in_maps(inputs)

    from concourse.bass_utils import run_bass_kernel_spmd

    res = run_bass_kernel_spmd(nc, in_maps, core_ids=list(range(NCORES)))
    _CACHE["last_result"] = res
    out = np.concatenate(
        [
            np.asarray(res.results[i]["outv"], dtype=np.float32).reshape(BPC, C, N)
            for i in range(NCORES)
        ],
        axis=0,
    )
    return out


# revision 70
# speedup vs baseline: 1.0066x; 1.0066x over previous
"""EnhancedAttentionModule Trainium2 kernel.

x: [16, 512, 4096] f32.  Module:
    pooled = mean_n(x)                      # [B, C]
    h  = relu(pooled @ w1.T + b1)           # [B, C/4]
    ca = sigmoid(h @ w2.T + b2)             # [B, C]  (channel attention)
    x_ca = x * ca[:, :, None]
    h2 = BN(w3 @ x_ca + b3); h2 = relu(h2)  # [B, C/4, N]
    sa = sigmoid(w4 @ h2 + b4)              # [B, 1, N] (spatial attention)
    out = x + x_ca * sa = x * (1 + ca*sa)

The problem is HBM-bound on a serial DMA resource (~360 GB/s modeled),
so both wire formats are reduced precision (harness tolerance 2e-2):
  - x is converted to float16 on the host before upload (halves load
    traffic; adds ~2.4e-4 relative error),
  - the output is written as float16 and upcast on the host during the
    gather (halves store traffic).
Per core: 8.4 MB read + 8.4 MB write + ~0.8 MB weights.

Device-side structure:
  - mean divisor folded into w1, BN folded into w3/bias (host); ca
    folded into the w3 matmul weights on device (w3e = w3Ti * ca) so
    x_ca is never materialized.
  - out = x * (1 + ca[c]*sa[n]): the rank-1 product ca (x) sa goes
    straight into PSUM via a K=1 matmul per [128,512] bank; the +1 and
    the multiply fuse into one DVE scalar_tensor_tensor per [128,1024]
    block ((s2raw + 1) * x -> f16), stored immediately (32 block
    stores/core).
  - for j==3 (both batches) and j==2 (batch 0) ACT instead evicts
    s2+1 to f16 SBUF (activation Copy, bias=1.0) and the otherwise
    idle Pool engine does the multiply -- GPSIMD cannot read PSUM on
    real HW, and this keeps DVE production ahead of the DMA drain.
  - each pooled sum is ONE DVE op: scalar_tensor_tensor computes the
    f16 pairwise add of the tile halves with accum_out producing the
    sum directly.  The last tile of each batch is pooled in quarters
    so the MLP can start right after its DMA lands; the MLP
    accumulates all 7 partial vectors via matmul linearity.  All
    large weights (w3Ti, w1nT, w2T, w4) ship as one f16 blob and the
    whole channel-MLP runs in f16.
  - sa pairs interleave with s2/multiply/store emission, each group
    one pair late so PE (in-order) never stalls on the pair's
    sigmoids; the sa PSUM ring is double-buffered (the MLP borrows a
    ps_h2 slot to free the bank).
  - weight DMAs are emitted after the first x-tile load so their
    HWDGE issue latency hides under the big transfer (back-to-back
    small transfers cannot keep the issue pipeline fed).
  - a dummy sigmoid at kernel start pulls both activation-table loads
    off the critical path.
  - matmuls run f16 (x-side) and float32r (s2/sa side), 1 cycle/row.

Sharding: data-parallel over batch. 8 cores x 2 batches each. Weights
replicated. No collectives.
"""

import numpy as np

B, C, N = 16, 512, 4096
CR = C // 4  # 128
P = 128      # partitions
NCORES = 8
BPC = B // NCORES        # batches per core = 2
CCH = C // P             # channel chunks per batch = 4
NB = N // 512            # 512-wide n blocks = 8
NH = N // 1024           # 1024-wide blocks = 4
BN_EPS = 1e-5

# f16 weight blob ([128, HBLOB])
_W3 = 0          # w3Ti as [p, j, m]: cols [0, 512)
_W1 = 512        # w1nT as [p, j, m]: cols [512, 1024)
_W2 = 1024       # w2T: cols [1024, 1536)
_W4H = 1536      # w4T: col 1536
HBLOB = 1537
# f32 weight blob ([128, FBLOB]): small per-partition vectors
_B1 = 0
_B3 = 1
_B2C = 2         # cols [2, 6)
_B4 = 6          # row 0 only
FBLOB = 7

_CACHE = {}


def _build(n_iter=1):
    import concourse.bacc as bacc
    import concourse.tile as tile
    from concourse import mybir

    f32 = mybir.dt.float32
    f32r = mybir.dt.float32r
    f16 = mybir.dt.float16
    AF = mybir.ActivationFunctionType
    AX = mybir.AxisListType
    ALU = mybir.AluOpType

    nc = bacc.Bacc(None)

    xs = nc.dram_tensor("xs", [BPC * C, N], f16, kind="ExternalInput")
    out = nc.dram_tensor("outv", [BPC * C, N], f16, kind="ExternalOutput")
    wbh_d = nc.dram_tensor("wblobh", [P, HBLOB], f16, kind="ExternalInput")
    wbf_d = nc.dram_tensor("wblobf", [P, FBLOB], f32, kind="ExternalInput")
    b2r_d = nc.dram_tensor("b2row", [1, C], f16, kind="ExternalInput")

    xs_t = xs.rearrange("(t p) n -> t p n", p=P)      # 8 tiles [128, 4096]
    out_t = out.rearrange("(t p) n -> t p n", p=P)

    with tile.TileContext(nc) as tc:
        with (
            tc.tile_pool(name="wpool", bufs=1) as wpool,
            tc.tile_pool(name="xpool", bufs=BPC * CCH) as xpool,
            tc.tile_pool(name="small", bufs=6) as small,
            tc.tile_pool(name="ppool", bufs=3) as ppool,
            tc.tile_pool(name="wefpool", bufs=2 * CCH) as wefpool,
            tc.tile_pool(name="h2spool", bufs=6) as h2spool,
            tc.tile_pool(name="sapool", bufs=2) as sapool,
            tc.tile_pool(name="opool", bufs=32) as opool,
            tc.tile_pool(name="ps_h2", bufs=2, space="PSUM") as ps_h2,
            tc.tile_pool(name="ps_sa", bufs=2, space="PSUM") as ps_sa,
            tc.tile_pool(name="ps_s2", bufs=2, space="PSUM") as ps_s2,
        ):
            wbh = wpool.tile([P, HBLOB], f16)
            wbf = wpool.tile([P, FBLOB], f32)
            b2r_sb = wpool.tile([1, C], f16)
            w3Ti_sb = wbh[:, _W3 : _W3 + 512].rearrange("p (j m) -> p j m", j=CCH)
            w1nT_sb = wbh[:, _W1 : _W1 + 512].rearrange("p (j m) -> p j m", j=CCH)
            w2T_sb = wbh[:, _W2 : _W2 + 512]
            b1_sb = wbf[:, _B1 : _B1 + 1]
            b3e_sb = wbf[:, _B3 : _B3 + 1]
            b2c_sb = wbf[:, _B2C : _B2C + CCH]
            b4_sb = wbf[0:1, _B4 : _B4 + 1]
            w4T_sb = wbh[:, _W4H : _W4H + 1]
            one1f = wpool.tile([1, 1], f32)
            nc.vector.memset(one1f, 1.0)
            one1_sb = wpool.tile([1, 1], f16)
            nc.vector.tensor_copy(one1_sb, one1f)
            # dummy sigmoid: forces the activation-table switch (~1.3us)
            # to happen at kernel start while ACT is idle, not mid-MLP
            dummy = wpool.tile([1, 1], f32)
            nc.scalar.activation(dummy, one1f, AF.Sigmoid)
            # sa|ca tiles: cols [0,N) = sa, [N,N+C) = ca_row.  Single
            # row: s2 is computed as the rank-1 product ca (x) sa by a
            # K=1 matmul and the +1 is folded into the final multiply
            # (scalar_tensor_tensor), so no ones rows are needed.
            sa_tiles = []
            for _b in range(BPC):
                sa_t = sapool.tile([1, N + C], f32r, tag="sa")
                sa_tiles.append(sa_t)

            def emit_weight_dmas():
                nc.sync.dma_start(out=wbh, in_=wbh_d[:, :])
                nc.sync.dma_start(out=wbf, in_=wbf_d[:, :])
                nc.sync.dma_start(out=b2r_sb, in_=b2r_d[:, :])

            for _it in range(n_iter):
                # ---- x loads, all emitted up front.  The LAST tile of
                # each batch is split into four quarter-column DMAs so
                # its pooled sum can complete right after the last byte.
                xts = []
                for b in range(BPC):
                    xt = []
                    for j in range(CCH):
                        t = xpool.tile([P, N], f16, tag="xt")
                        xt.append(t)
                        if b == 0 and j == CCH - 1:
                            for q in range(4):
                                nc.sync.dma_start(
                                    out=t[:, q * 1024 : (q + 1) * 1024],
                                    in_=xs_t[j][:, q * 1024 : (q + 1) * 1024],
                                )
                        else:
                            nc.sync.dma_start(out=t, in_=xs_t[b * CCH + j])
                        if _it == 0 and b == 0 and j == 0:
                            emit_weight_dmas()
                    xts.append(xt)

                # ---- pooled sums, two-stage (DVE f16 pair-adds at 2x
                # rate, then a short reduce).  Batch 0 stage 2 on ACT
                # (idle during loads); batch 1 fully on DVE so it isn't
                # queued behind batch 0's ACT chain.  Emitted for both
                # batches here so the reduces track the DMA arrivals.
                parts_by_b = []
                with nc.allow_low_precision(reason="f16 pairwise add; accum f32"):
                    for b in range(BPC):
                        xt = xts[b]
                        parts = []
                        for j in range(CCH - 1):
                            t = xt[j]
                            h1 = ppool.tile([P, 2048], f16, tag="h1")
                            pj = small.tile([P, 1], f16, tag=f"pool_{b}_{j}")
                            nc.vector.scalar_tensor_tensor(
                                h1, t[:, :2048], 0.0, t[:, 2048:],
                                op0=ALU.add, op1=ALU.add, accum_out=pj,
                            )
                            parts.append((j, pj))
                        t = xt[CCH - 1]
                        for q in range(4):
                            qv = t[:, q * 1024 : (q + 1) * 1024]
                            hq = ppool.tile([P, 512], f16, tag="hq")
                            pq = small.tile([P, 1], f16, tag=f"poolq_{b}_{q}")
                            nc.vector.scalar_tensor_tensor(
                                hq, qv[:, :512], 0.0, qv[:, 512:],
                                op0=ALU.add, op1=ALU.add, accum_out=pq,
                            )
                            parts.append((CCH - 1, pq))
                        parts_by_b.append(parts)

                for b in range(BPC):
                    xt = xts[b]
                    # ---- channel attention MLP ----
                    # the MLP psum borrows a ps_h2 ring slot (same shape);
                    # it is consumed well before the second h2 block needs
                    # the slot back, and this frees a PSUM bank so the sa
                    # ring can double-buffer (sa-mm(k+1) must not wait on
                    # sigmoid(k))
                    psum_hca = ps_h2.tile([P, 512], f32, tag="ph2")
                    psum_h = psum_hca[:, 0:1]
                    psum_ca = psum_hca[:, 4:8]
                    parts = parts_by_b[b]
                    for k, (j, pv) in enumerate(parts):
                        nc.tensor.matmul(
                            psum_h,
                            lhsT=w1nT_sb[:, j, :],
                            rhs=pv,
                            start=(k == 0),
                            stop=(k == len(parts) - 1),
                        )
                    h_sb = small.tile([P, 1], f16, tag="h")
                    nc.scalar.activation(h_sb, psum_h, AF.Relu, bias=b1_sb)

                    # ca as per-partition columns [P, CCH] (for the w3 fold)
                    for j in range(CCH):
                        nc.tensor.matmul(
                            psum_ca[:, j : j + 1],
                            lhsT=w2T_sb[:, j * P : (j + 1) * P],
                            rhs=h_sb,
                            start=True,
                            stop=True,
                        )
                    ca_sb = small.tile([P, CCH], f32, tag="ca")
                    for j in range(CCH):
                        nc.scalar.activation(
                            ca_sb[:, j : j + 1],
                            psum_ca[:, j : j + 1],
                            AF.Sigmoid,
                            bias=b2c_sb[:, j : j + 1],
                        )

                    # ca as an augmented row pair: row0 = sigmoid(h@w2T + b2)
                    psum_car = ps_sa.tile([1, C], f32, tag="psa")
                    nc.tensor.matmul(
                        psum_car, lhsT=h_sb, rhs=w2T_sb, start=True, stop=False
                    )
                    nc.tensor.matmul(
                        psum_car, lhsT=one1_sb, rhs=b2r_sb, start=False, stop=True
                    )
                    ca2_sb = sa_tiles[b][:, N : N + C]
                    nc.scalar.activation(ca2_sb[0:1, :], psum_car, AF.Sigmoid)

                    # ---- fold ca into w3 (ACT: out = Copy(in * scale)) ----
                    w3e = []
                    for j in range(CCH):
                        we = wefpool.tile([P, CR], f16, tag="w3e")
                        nc.scalar.activation(
                            we, w3Ti_sb[:, j, :], AF.Copy, scale=ca_sb[:, j : j + 1]
                        )
                        w3e.append(we)

                    # ---- spatial attention + output, interleaved ----
                    sa_sb = sa_tiles[b]

                    def emit_s2_mul_store(nh, b=b, sa_sb=sa_sb, xt=xt,
                                          ca2_sb=ca2_sb):
                        lo = nh * 1024
                        for j in range(CCH):
                            psum_s2 = ps_s2.tile([P, 1024], f32, tag="ps2")
                            for hh in range(2):
                                o = lo + hh * 512
                                nc.tensor.matmul(
                                    psum_s2[:, hh * 512 : (hh + 1) * 512],
                                    lhsT=ca2_sb[:, j * P : (j + 1) * P],
                                    rhs=sa_sb[:, o : o + 512],
                                    start=True,
                                    stop=True,
                                )
                            ob = opool.tile([P, 1024], f16, tag="ob")
                            # out = (ca*sa + 1) * x.  GPSIMD/Pool cannot
                            # read PSUM on real HW, so DVE multiplies from
                            # PSUM directly; for j==3 ACT evicts s2+1 to
                            # f16 SBUF and the otherwise-idle Pool engine
                            # does the multiply.
                            if j == CCH - 1 or (b == 0 and j == CCH - 2):
                                s2f = ppool.tile([P, 1024], f16, tag="s2f")
                                nc.scalar.activation(
                                    s2f, psum_s2, AF.Copy, bias=1.0
                                )
                                nc.gpsimd.tensor_mul(
                                    ob, s2f, xt[j][:, lo : lo + 1024]
                                )
                            elif b == 1 and nh == NH - 1 and j == CCH - 2:
                                # tail-critical block: multiply and store in
                                # 512-halves so the first store issues while
                                # the second half multiplies, and the final
                                # transfer is half-size
                                for hh in range(2):
                                    o = hh * 512
                                    nc.vector.scalar_tensor_tensor(
                                        ob[:, o : o + 512],
                                        psum_s2[:, o : o + 512],
                                        1.0,
                                        xt[j][:, lo + o : lo + o + 512],
                                        op0=ALU.add,
                                        op1=ALU.mult,
                                    )
                                    nc.sync.dma_start(
                                        out=out_t[b * CCH + j][
                                            :, lo + o : lo + o + 512
                                        ],
                                        in_=ob[:, o : o + 512],
                                    )
                                continue
                            else:
                                nc.vector.scalar_tensor_tensor(
                                    ob,
                                    psum_s2,
                                    1.0,
                                    xt[j][:, lo : lo + 1024],
                                    op0=ALU.add,
                                    op1=ALU.mult,
                                )
                            nc.sync.dma_start(
                                out=out_t[b * CCH + j][:, lo : lo + 1024], in_=ob
                            )

                    for nb in range(NB):
                        psum_h2 = ps_h2.tile([P, 512], f32, tag="ph2")
                        for j in range(CCH):
                            nc.tensor.matmul(
                                psum_h2,
                                lhsT=w3e[j],
                                rhs=xt[j][:, nb * 512 : (nb + 1) * 512],
                                start=(j == 0),
                                stop=(j == CCH - 1),
                            )
                        h2s = h2spool.tile([P, 512], f16, tag="h2s")
                        nc.scalar.activation(h2s, psum_h2, AF.Relu, bias=b3e_sb)
                        psum_sa = ps_sa.tile([1, 512], f32, tag="psa")
                        nc.tensor.matmul(
                            psum_sa, lhsT=w4T_sb, rhs=h2s, start=True, stop=True
                        )
                        nc.scalar.activation(
                            sa_sb[0:1, nb * 512 : (nb + 1) * 512],
                            psum_sa,
                            AF.Sigmoid,
                            bias=b4_sb,
                        )
                        # s2/mult/store groups are emitted ONE PAIR LATE:
                        # group g's s2 matmuls depend on pair g's sigmoids,
                        # so emitting them right after pair g stalls PE on
                        # ACT; one pair of h2 work in between hides it.
                        if nb % 2 == 1 and nb >= 3:
                            emit_s2_mul_store((nb - 3) // 2)
                    emit_s2_mul_store(NH - 1)

    nc.finalize()
    return nc


def _get_nc(n_iter=1):
    key = ("nc", n_iter)
    if key not in _CACHE:
        _CACHE[key] = _build(n_iter)
    return _CACHE[key]


def _make_in_maps(inputs):
    x = np.asarray(inputs["x"], dtype=np.float32)
    w1 = np.asarray(inputs["w1"], dtype=np.float32)
    b1 = np.asarray(inputs["b1"], dtype=np.float32)
    w2 = np.asarray(inputs["w2"], dtype=np.float32)
    b2 = np.asarray(inputs["b2"], dtype=np.float32)
    w3 = np.asarray(inputs["w3"], dtype=np.float32)
    b3 = np.asarray(inputs["b3"], dtype=np.float32)
    bn_gamma = np.asarray(inputs["bn_gamma"], dtype=np.float32)
    bn_beta = np.asarray(inputs["bn_beta"], dtype=np.float32)
    bn_mean = np.asarray(inputs["bn_mean"], dtype=np.float32)
    bn_var = np.asarray(inputs["bn_var"], dtype=np.float32)
    w4 = np.asarray(inputs["w4"], dtype=np.float32)
    b4 = np.asarray(inputs["b4"], dtype=np.float32)

    # ---- host-side weight folding (tiny) + f16 wire conversion ----
    inv = bn_gamma / np.sqrt(bn_var + BN_EPS)                   # [CR]
    w1nT = (w1.T / float(N)).reshape(CCH, P, CR).transpose(1, 0, 2)
    w3Ti = (w3.T * inv[None, :]).reshape(CCH, P, CR).transpose(1, 0, 2)
    b3e = b3 * inv + bn_beta - bn_mean * inv

    x16 = np.ascontiguousarray(x.astype(np.float16))
    wbh = np.zeros((P, HBLOB), np.float16)
    wbh[:, _W3 : _W3 + 512] = w3Ti.reshape(P, 512).astype(np.float16)
    wbh[:, _W1 : _W1 + 512] = w1nT.reshape(P, 512).astype(np.float16)
    wbh[:, _W2 : _W2 + 512] = w2.T.astype(np.float16)            # [CR->P, C]
    wbh[:, _W4H] = w4.reshape(CR).astype(np.float16)
    wbf = np.zeros((P, FBLOB), np.float32)
    wbf[:, _B1] = b1
    wbf[:, _B3] = b3e
    wbf[:, _B2C : _B2C + CCH] = b2.reshape(CCH, P).T
    wbf[0, _B4] = b4[0]
    b2row = np.ascontiguousarray(b2.reshape(1, C).astype(np.float16))

    in_maps = []
    for i in range(NCORES):
        in_maps.append(
            {
                "xs": x16[i * BPC : (i + 1) * BPC].reshape(BPC * C, N),
                "wblobh": wbh,
                "wblobf": wbf,
                "b2row": b2row,
            }
        )
    return in_maps


def kernel(**inputs):
    nc = _get_nc()
    in_maps = _make_in_maps(inputs)

    from concourse.bass_utils import run_bass_kernel_spmd

    res = run_bass_kernel_spmd(nc, in_maps, core_ids=list(range(NCORES)))
    _CACHE["last_result"] = res
    out = np.concatenate(
        [
            np.asarray(res.results[i]["outv"], dtype=np.float32).reshape(BPC, C, N)
            for i in range(NCORES)
        ],
        axis=0,
    )
    return out
